# revision 29
# baseline (speedup 1.0000x reference)
"""Trainium2 Bass kernel for nn_Encoder_50852412785097 (sparse_attention).

Math (validated against the jax reference):
  Per (b, h):
    Q = X wQ_h / 8, K = X wK_h, V = X wV_h      (X = inputs[b], [S, D])
    es = Q K^T                                   (causal, mask = -20)
    ex1 = exp(es), den = sum_j ex1
    rrn_ij = 1 - cumsum_j(ex1)/den               (inclusive cumsum)
    z = th2 * (t_i - t_j) * rrn                  (z in [0, 0.33] since
                                                  th2 = theta^2 ~ 3.3e-6)
    decay = exp(-z) ~= (SC*(t_i - t_j)*rrn - H)^2   (minimax quadratic;
                                                  SC = sqrt(c2)*th2 folded
                                                  into the timestamps host-side)
    u = exp(es * decay), den2 = sum_j u
    out_h = ((u @ V) / den2) @ wO_h
  out[b] = sum_h out_h

Sharding: 16 (b, h) pairs over 8 cores -> core c handles b = c//4,
heads {2*(c%4), 2*(c%4)+1}. Weights replicated; host sums the 4 partial
outputs per batch.

Implementation highlights vs the previous version (298us):
  - a custom DVE op (registered at build time into dve_ops.OPS) computes
    decay' = ((t_i' - t_j')*scan_subtract(ex1*rden, init=1) - H)^2 in ONE
    1.07-cyc/elem pass: replaces stock scan (2.1-2.6 cyc/elem) + stt +
    ACT exp(decay) + gpsimd multiply.
  - fp16 everywhere off the PE accumulators (values are all in [-21, 2.5]):
    DVE 2x/4x modes for mults/copies, half SBUF.
  - per-(h,ti) elementwise passes are single full-width instructions
    (accum_out gives den/den2 directly, no per-chunk reduces).
  - projections for both heads packed into one [64,128] weight matrix ->
    qt/kt stacked on partitions, v interleaved; 1024-wide QK matmuls.
"""

import os
import sys

import numpy as np

B, S, H, D = 2, 2048, 8, 64
P = 128
NT = S // P  # 16 row tiles
NH = 2  # heads per core
NCORES = 8
MASK_VAL = -20.0

# exp(-z) ~= C2P*(z - HP)^2 on z in [0, 0.3291]; max err 2.88e-3
C2P = 0.213271
HP = 2.162264
# 3-param variant: C2F*(z-HF)^2 + KF, max err 1.58e-4
C2F = 0.425335
HF = 1.165113
KF = 0.422454

TH2 = np.float64(3.2929192e-06)  # theta^2 (theta fixed by the seed); the
# actual value is recomputed on the host from the theta input each call.


def _import_concourse():
    try:
        import concourse.bass  # noqa: F401
    except ImportError:
        for p in ("/opt/trn_rl_repo", "/root/.axon_site/_ro/trn_rl_repo"):
            if os.path.isdir(p) and p not in sys.path:
                sys.path.insert(0, p)
        import concourse.bass  # noqa: F401


def _register_fused_op(fit3):
    """Register the fused decay op in dve_ops.OPS (idempotent)."""
    _import_concourse()
    from concourse import dve_ops
    from concourse.dve_spec import (Spec, Src0, Src1, C0, C1, C2, One,
                                    AluOp, lower, sq, scan, _has_src1)
    from concourse.dve_uop import DveOpSpec

    name = "SPARSE_DECAY_FUSED"
    for op in dve_ops.OPS:
        if op.name == name:
            return op
    body = sq((C0 - Src0) * scan(AluOp.SUBTRACT, Src1 * C1, init=One) - C2)

    def ref(in0, in1, s0, s1, imm2):
        cum = np.cumsum(in1.astype(np.float32), axis=-1)
        return (((s0 - in0.astype(np.float32))
                 * (1.0 - cum * s1) - imm2) ** 2)

    spec = Spec(body=body, reference=ref)
    row = dve_ops._CUSTOM_DVE_ROW_BASE + len(dve_ops.OPS)
    assert row < 0x20
    dve_ops._SUB_OPCODE_FOR_NAME[name] = row
    shas = {}
    for ver in ("v3", "v4"):
        try:
            s = DveOpSpec(name=name, opcode=row, uops=lower(spec, ver=ver),
                          rd1_en=_has_src1(spec))
            shas[ver] = s.sha(ver)
        except Exception:
            pass
    op = dve_ops.DveOp(name, spec, subdim=False, uops_sha=shas)
    dve_ops.OPS.append(op)
    dve_ops.CUSTOM_DVE_SPECS[name] = spec
    return op


def build_nc():
    """Build the SPMD single-core program (same on all 8 cores)."""
    _import_concourse()
    import concourse.bass as bass
    import concourse.bacc as bacc
    from concourse import mybir
    from concourse.tile import TileContext

    fused_op = _register_fused_op(False)

    f32 = mybir.dt.float32
    f16 = mybir.dt.float16
    Alu = mybir.AluOpType
    Act = mybir.ActivationFunctionType

    h_imm = HP * float(np.sqrt(C2P))

    nc = bacc.Bacc("TRN2", target_bir_lowering=False, debug=False)

    # --- external I/O (per core) ---
    xT_h = nc.dram_tensor("xT", [D, S], f16, kind="ExternalInput")    # X^T
    tsj_h = nc.dram_tensor("tsj", [1, S], f32, kind="ExternalInput")  # t_j' row
    tsi_h = nc.dram_tensor("tsi", [P, NT], f32, kind="ExternalInput")  # t_i' cols
    wq_h = nc.dram_tensor("wq", [D, NH * D], f16, kind="ExternalInput")
    wk_h = nc.dram_tensor("wk", [D, NH * D], f16, kind="ExternalInput")
    wv_h = nc.dram_tensor("wv", [D, NH * D], f16, kind="ExternalInput")
    wo_h = nc.dram_tensor("wo", [D, NH * D], f16, kind="ExternalInput")
    # unnormalized per-head outputs + den2; normalization happens host-side
    y_h = nc.dram_tensor("y", [S, NH * D], f32, kind="ExternalOutput")
    d2_h = nc.dram_tensor("d2", [P, NT * NH], f32, kind="ExternalOutput")

    # --- NEFF-embedded constants ---
    mask_np = np.triu(np.ones((P, P), np.float32), k=1) * np.float32(MASK_VAL)
    mask_dram = nc.inline_tensor(mask_np, name="maskc")
    ident_dram = nc.inline_tensor(np.eye(P, dtype=np.float16), name="identc")

    with TileContext(nc) as tc:
        from contextlib import ExitStack

        with ExitStack() as ctx:
            consts = ctx.enter_context(tc.tile_pool(name="consts", bufs=1))

            # DMA inputs into staging tiles, then stage through a single
            # compute engine so downstream consumers wait on ONE semaphore.
            def load(shape, handle_ap, via, name, dt=f32, sdt=f32):
                stage = consts.tile(shape, sdt, tag=f"stg_{name}")
                nc.gpsimd.dma_start(out=stage, in_=handle_ap)
                dst = consts.tile(shape, dt, tag=name)
                via(dst, stage)
                return dst

            # PE-consumed: staged via DVE
            mask = load([P, P], mask_dram[:, :], nc.vector.tensor_copy, "mask")
            identf = load([P, P], ident_dram[:, :], nc.vector.tensor_copy,
                          "identf", dt=f16, sdt=f16)
            xT = load([D, S], xT_h[:, :], nc.vector.tensor_copy, "xT",
                      dt=f16, sdt=f16)
            wq = load([D, NH * D], wq_h[:, :], nc.vector.tensor_copy, "wq",
                      dt=f16, sdt=f16)
            wk = load([D, NH * D], wk_h[:, :], nc.vector.tensor_copy, "wk",
                      dt=f16, sdt=f16)
            wv = load([D, NH * D], wv_h[:, :], nc.vector.tensor_copy, "wv",
                      dt=f16, sdt=f16)
            wo = load([D, NH * D], wo_h[:, :], nc.vector.tensor_copy, "wo",
                      dt=f16, sdt=f16)

            # DVE-consumed: staged via ACT
            tsj_ap = tsj_h[:, :]
            tsj_b = bass.AP(
                tensor=tsj_ap.tensor, offset=tsj_ap.offset,
                ap=[[0, P], list(tsj_ap.ap[-1])],
            )
            tsj = load([P, S], tsj_b, nc.scalar.copy, "tsj")
            tsi = load([P, NT], tsi_h[:, :], nc.scalar.copy, "tsi")

            # --- projections: qt, kt [128(2h*64d), S]; v2 [128s, NT*128(2h*64e)] ---
            qt = consts.tile([P, S], f16)
            kt = consts.tile([P, S], f16)
            v2 = consts.tile([P, NT * P], f16)
            with tc.tile_pool(name="psetup", bufs=2, space="PSUM") as psetup:
                for w8, dst in ((wq, qt), (wk, kt)):
                    for sc in range(S // 1024):
                        pq = psetup.tile([P, 1024], f32, tag="ps")
                        for j in (0, 512):
                            nc.tensor.matmul(
                                pq[:, j:j + 512], lhsT=w8,
                                rhs=xT[:, 1024 * sc + j:1024 * sc + j + 512],
                                start=True, stop=True)
                        nc.scalar.copy(dst[:, 1024 * sc:1024 * (sc + 1)], pq)
                for g in range(2):  # 8 s-tiles per round -> one [P,1024] psum
                    pv = psetup.tile([P, 8 * P], f32, tag="ps")
                    for q in range(8):
                        st = 8 * g + q
                        nc.tensor.matmul(pv[:, q * P:(q + 1) * P],
                                         lhsT=xT[:, P * st:P * (st + 1)],
                                         rhs=wv, start=True, stop=True)
                    nc.scalar.copy(v2[:, g * 8 * P:(g + 1) * 8 * P], pv)

            # --- main pipeline ---
            d2all = consts.tile([P, NT * NH], f32, tag="d2all")

            work = ctx.enter_context(tc.tile_pool(name="work", bufs=4))
            small = ctx.enter_context(tc.tile_pool(name="small", bufs=6))
            ppe = ctx.enter_context(tc.tile_pool(name="ppe", bufs=2,
                                                 space="PSUM"))
            ppt = ctx.enter_context(tc.tile_pool(name="ppt", bufs=2,
                                                 space="PSUM"))
            pprT = ctx.enter_context(tc.tile_pool(name="pprT", bufs=1,
                                                  space="PSUM"))
            ppo = ctx.enter_context(tc.tile_pool(name="ppo", bufs=1,
                                                 space="PSUM"))

            CH = 1024  # QK chunk width (PSUM tile)

            for ti in range(NT):
                W = P * (ti + 1)
                nch = (W + CH - 1) // CH
                po2 = ppo.tile([P, NH, D], f32)
                for h in range(NH):
                    hs = slice(D * h, D * (h + 1))
                    # QK scores -> PSUM chunks -> es fp16 in SBUF
                    es = work.tile([P, S], f16, tag="es")
                    qrow = qt[hs, P * ti:P * (ti + 1)]
                    cpy = nc.vector.tensor_copy if h == 0 else nc.scalar.copy
                    for c in range(nch):
                        lo, hi = CH * c, min(W, CH * (c + 1))
                        pe = ppe.tile([P, CH], f32, tag="pe")
                        j0 = lo
                        while j0 < hi:
                            j1 = min(hi, j0 + 512)
                            nc.tensor.matmul(pe[:, j0 - lo:j1 - lo],
                                             lhsT=qrow, rhs=kt[hs, j0:j1],
                                             start=True, stop=True)
                            j0 = j1
                        cols = hi - lo
                        if hi == W:
                            if cols > P:
                                cpy(es[:, lo:hi - P], pe[:, :cols - P])
                            nc.vector.tensor_add(
                                es[:, W - P:W], pe[:, cols - P:cols], mask)
                        else:
                            cpy(es[:, lo:hi], pe[:, :cols])

                    # ex1 = exp(es), den via accum (one full-width instr)
                    ex1 = work.tile([P, S], f16, tag="ex1")
                    den = small.tile([P, 1], f32, tag="den")
                    nc.scalar.activation(ex1[:, :W], es[:, :W], Act.Exp,
                                         accum_out=den)
                    rden = small.tile([P, 1], f32, tag="rden")
                    nc.vector.reciprocal(rden, den)

                    # decay' = ((t_i'-t_j')*(1-cum*rden) - H)^2  [custom DVE]
                    dz = work.tile([P, S], f16, tag="dz")
                    nc.vector._custom_dve(
                        fused_op, out=dz[:, :W], in0=tsj[:, :W],
                        in1=ex1[:, :W], s0=tsi[:, ti:ti + 1], s1=rden,
                        imm2=h_imm)

                    # w = es * decay'   (fp16 2x)
                    w = work.tile([P, S], f16, tag="w")
                    nc.vector.tensor_mul(w[:, :W], es[:, :W], dz[:, :W])

                    # u = exp(w), den2 accumulated straight into d2all
                    u = work.tile([P, S], f16, tag="u")
                    nc.scalar.activation(u[:, :W], w[:, :W], Act.Exp,
                                         accum_out=d2all[:, ti * NH + h:
                                                         ti * NH + h + 1])

                    # AV: transpose u blocks (8 per PSUM tile), accumulate
                    prT = pprT.tile([D, P], f32, tag="prT")
                    njb = ti + 1
                    for g0 in range(0, njb, 8):
                        gn = min(8, njb - g0)
                        uT8 = small.tile([P, 8 * P], f16, tag="uT8")
                        pt = ppt.tile([P, 8 * P], f16, tag="pt")
                        for q in range(gn):
                            nc.tensor.transpose(
                                pt[:, q * P:(q + 1) * P],
                                u[:, (g0 + q) * P:(g0 + q + 1) * P], identf)
                        nc.vector.tensor_copy(uT8[:, :gn * P], pt[:, :gn * P])
                        for q in range(gn):
                            jb = g0 + q
                            nc.tensor.matmul(
                                prT,
                                lhsT=v2[:, jb * P + D * h: jb * P + D * (h + 1)],
                                rhs=uT8[:, q * P:(q + 1) * P],
                                start=(jb == 0), stop=(jb == ti))
                    rT = small.tile([D, P], f16, tag="rT")
                    nc.vector.tensor_copy(rT, prT)
                    nc.tensor.matmul(po2[:, h, :], lhsT=rT, rhs=wo[:, hs],
                                     start=True, stop=True)

                # unnormalized head outputs -> SBUF -> DRAM
                ys = small.tile([P, NH * D], f32, tag="ys")
                nc.vector.tensor_copy(ys, po2[:, :, :])
                nc.sync.dma_start(out=y_h[P * ti:P * (ti + 1), :], in_=ys)
            nc.sync.dma_start(out=d2_h[:, :], in_=d2all)

    if not nc.is_finalized():
        nc.finalize()
    return nc


_NC_CACHE = {}

KERNEL_FLAGS = {}


def _get_nc():
    key = tuple(sorted(KERNEL_FLAGS.items()))
    if key not in _NC_CACHE:
        _NC_CACHE[key] = build_nc(**KERNEL_FLAGS)
    return _NC_CACHE[key]


def make_in_maps(inputs, timestamp, wQ, wK, wV, wO, theta):
    x = np.asarray(inputs, np.float32)
    t64 = np.asarray(timestamp).astype(np.float64)
    wQ = np.asarray(wQ, np.float32)
    wK = np.asarray(wK, np.float32)
    wV = np.asarray(wV, np.float32)
    wO = np.asarray(wO, np.float32)
    th2 = float(np.float64(np.asarray(theta, np.float64)[0, 0]) ** 2)
    sc = th2 * float(np.sqrt(C2P))
    tp = (t64 * sc).astype(np.float32)  # prescaled timestamps

    in_maps = []
    for c in range(NCORES):
        b = c // 4
        h0 = NH * (c % 4)
        in_maps.append({
            "xT": np.ascontiguousarray(x[b].T.astype(np.float16)),
            "tsj": np.ascontiguousarray(tp[b][None, :]),
            "tsi": np.ascontiguousarray(tp[b].reshape(NT, P).T),
            "wq": np.ascontiguousarray(
                (np.concatenate([wQ[h0], wQ[h0 + 1]], axis=1)
                 * 0.125).astype(np.float16)),
            "wk": np.ascontiguousarray(np.concatenate(
                [wK[h0], wK[h0 + 1]], axis=1).astype(np.float16)),
            "wv": np.ascontiguousarray(np.concatenate(
                [wV[h0], wV[h0 + 1]], axis=1).astype(np.float16)),
            "wo": np.ascontiguousarray(np.concatenate(
                [wO[h0 * D:(h0 + 1) * D], wO[(h0 + 1) * D:(h0 + 2) * D]],
                axis=1).astype(np.float16)),
        })
    return in_maps


def kernel(inputs, timestamp, wQ, wK, wV, wO, theta, _trace=False,
           _trace_kwargs=None):
    _import_concourse()
    from concourse.bass_utils import run_bass_kernel_spmd

    nc = _get_nc()
    in_maps = make_in_maps(inputs, timestamp, wQ, wK, wV, wO, theta)
    res = run_bass_kernel_spmd(
        nc, in_maps, list(range(NCORES)),
        trace=_trace, **(_trace_kwargs or {}),
    )
    out = np.zeros((B, S, D), np.float32)
    for c in range(NCORES):
        yp = res.results[c]["y"]  # [S, NH*D] unnormalized
        d2 = res.results[c]["d2"]  # [P, NT*NH]
        den2 = d2.reshape(P, NT, NH).transpose(1, 0, 2).reshape(S, NH)
        out[c // 4] += (yp[:, :D] / den2[:, 0:1]
                        + yp[:, D:] / den2[:, 1:2])
    if _trace:
        return out, res
    return out


if __name__ == "__main__":
    nc = build_nc()
    print("built ok")


# revision 33
# speedup vs baseline: 1.0290x; 1.0290x over previous
"""Trainium2 Bass kernel for nn_Encoder_50852412785097 (sparse_attention).

Math (validated against the jax reference):
  Per (b, h):
    Q = X wQ_h / 8, K = X wK_h, V = X wV_h      (X = inputs[b], [S, D])
    es = Q K^T                                   (causal, mask = -20)
    ex1 = exp(es), den = sum_j ex1
    rrn_ij = 1 - cumsum_j(ex1)/den               (inclusive cumsum)
    z = th2 * (t_i - t_j) * rrn                  (z in [0, 0.33] since
                                                  th2 = theta^2 ~ 3.3e-6)
    decay = exp(-z) ~= (SC*(t_i - t_j)*rrn - H)^2   (minimax quadratic;
                                                  SC = sqrt(c2)*th2 folded
                                                  into the timestamps host-side)
    u = exp(es * decay), den2 = sum_j u
    out_h = ((u @ V) / den2) @ wO_h
  out[b] = sum_h out_h

Sharding: 16 (b, h) pairs over 8 cores -> core c handles b = c//4,
heads {2*(c%4), 2*(c%4)+1}. Weights replicated; host sums the 4 partial
outputs per batch.

Implementation highlights vs the previous version (298us):
  - a custom DVE op (registered at build time into dve_ops.OPS) computes
    decay' = ((t_i' - t_j')*scan_subtract(ex1*rden, init=1) - H)^2 in ONE
    1.07-cyc/elem pass: replaces stock scan (2.1-2.6 cyc/elem) + stt +
    ACT exp(decay) + gpsimd multiply.
  - fp16 everywhere off the PE accumulators (values are all in [-21, 2.5]):
    DVE 2x/4x modes for mults/copies, half SBUF.
  - per-(h,ti) elementwise passes are single full-width instructions
    (accum_out gives den/den2 directly, no per-chunk reduces).
  - projections for both heads packed into one [64,128] weight matrix ->
    qt/kt stacked on partitions, v interleaved; 1024-wide QK matmuls.
"""

import os
import sys

import numpy as np

B, S, H, D = 2, 2048, 8, 64
P = 128
NT = S // P  # 16 row tiles
NH = 2  # heads per core
NCORES = 8
MASK_VAL = -20.0

# exp(-z) ~= C2P*(z - HP)^2 on z in [0, 0.3291]; max err 2.88e-3
C2P = 0.213271
HP = 2.162264
# 3-param variant: C2F*(z-HF)^2 + KF, max err 1.58e-4
C2F = 0.425335
HF = 1.165113
KF = 0.422454

TH2 = np.float64(3.2929192e-06)  # theta^2 (theta fixed by the seed); the
# actual value is recomputed on the host from the theta input each call.


def _import_concourse():
    try:
        import concourse.bass  # noqa: F401
    except ImportError:
        for p in ("/opt/trn_rl_repo", "/root/.axon_site/_ro/trn_rl_repo"):
            if os.path.isdir(p) and p not in sys.path:
                sys.path.insert(0, p)
        import concourse.bass  # noqa: F401


def _register_fused_op(fit3):
    """Register the fused decay op in dve_ops.OPS (idempotent)."""
    _import_concourse()
    from concourse import dve_ops
    from concourse.dve_spec import (Spec, Src0, Src1, C0, C1, C2, One,
                                    AluOp, lower, sq, scan, _has_src1)
    from concourse.dve_uop import DveOpSpec

    name = "SPARSE_DECAY_FUSED"
    for op in dve_ops.OPS:
        if op.name == name:
            return op
    body = sq((C0 - Src0) * scan(AluOp.SUBTRACT, Src1 * C1, init=One) - C2)

    def ref(in0, in1, s0, s1, imm2):
        cum = np.cumsum(in1.astype(np.float32), axis=-1)
        return (((s0 - in0.astype(np.float32))
                 * (1.0 - cum * s1) - imm2) ** 2)

    spec = Spec(body=body, reference=ref)
    row = dve_ops._CUSTOM_DVE_ROW_BASE + len(dve_ops.OPS)
    assert row < 0x20
    dve_ops._SUB_OPCODE_FOR_NAME[name] = row
    shas = {}
    for ver in ("v3", "v4"):
        try:
            s = DveOpSpec(name=name, opcode=row, uops=lower(spec, ver=ver),
                          rd1_en=_has_src1(spec))
            shas[ver] = s.sha(ver)
        except Exception:
            pass
    op = dve_ops.DveOp(name, spec, subdim=False, uops_sha=shas)
    dve_ops.OPS.append(op)
    dve_ops.CUSTOM_DVE_SPECS[name] = spec
    return op


def build_nc():
    """Build the SPMD single-core program (same on all 8 cores)."""
    _import_concourse()
    import concourse.bass as bass
    import concourse.bacc as bacc
    from concourse import mybir
    from concourse.tile import TileContext

    fused_op = _register_fused_op(False)

    f32 = mybir.dt.float32
    f16 = mybir.dt.float16
    Alu = mybir.AluOpType
    Act = mybir.ActivationFunctionType

    h_imm = HP * float(np.sqrt(C2P))

    nc = bacc.Bacc("TRN2", target_bir_lowering=False, debug=False)

    # --- external I/O (per core) ---
    xT_h = nc.dram_tensor("xT", [D, S], f16, kind="ExternalInput")    # X^T
    tsj_h = nc.dram_tensor("tsj", [1, S], f32, kind="ExternalInput")  # t_j' row
    tsi_h = nc.dram_tensor("tsi", [P, NT], f32, kind="ExternalInput")  # t_i' cols
    wq_h = nc.dram_tensor("wq", [D, NH * D], f16, kind="ExternalInput")
    wk_h = nc.dram_tensor("wk", [D, NH * D], f16, kind="ExternalInput")
    wv_h = nc.dram_tensor("wv", [D, NH * D], f16, kind="ExternalInput")
    wo_h = nc.dram_tensor("wo", [D, NH * D], f16, kind="ExternalInput")
    # unnormalized per-head outputs + den2; normalization happens host-side
    y_h = nc.dram_tensor("y", [S, NH * D], f32, kind="ExternalOutput")
    d2_h = nc.dram_tensor("d2", [P, NT * NH], f32, kind="ExternalOutput")

    # --- NEFF-embedded constants ---
    # mask512: zeros with the strictly-upper causal mask in the last 128
    # cols; matmul'd (via identity weights) into PSUM before the final QK
    # matmul of each row so the diagonal block lands pre-masked.
    mask512_np = np.zeros((P, 512), np.float16)
    mask512_np[:, 384:] = (np.triu(np.ones((P, P), np.float32), k=1)
                           * np.float32(MASK_VAL)).astype(np.float16)
    mask_dram = nc.inline_tensor(mask512_np, name="maskc")
    ident_dram = nc.inline_tensor(np.eye(P, dtype=np.float16), name="identc")

    with TileContext(nc) as tc:
        from contextlib import ExitStack

        with ExitStack() as ctx:
            consts = ctx.enter_context(tc.tile_pool(name="consts", bufs=1))

            # DMA inputs into staging tiles, then stage through a single
            # compute engine so downstream consumers wait on ONE semaphore.
            def load(shape, handle_ap, via, name, dt=f32, sdt=f32):
                stage = consts.tile(shape, sdt, tag=f"stg_{name}")
                nc.gpsimd.dma_start(out=stage, in_=handle_ap)
                dst = consts.tile(shape, dt, tag=name)
                via(dst, stage)
                return dst

            # PE-consumed: staged via DVE
            mask = load([P, 512], mask_dram[:, :], nc.vector.tensor_copy,
                        "mask", dt=f16, sdt=f16)
            identf = load([P, P], ident_dram[:, :], nc.vector.tensor_copy,
                          "identf", dt=f16, sdt=f16)
            xT = load([D, S], xT_h[:, :], nc.vector.tensor_copy, "xT",
                      dt=f16, sdt=f16)
            wq = load([D, NH * D], wq_h[:, :], nc.vector.tensor_copy, "wq",
                      dt=f16, sdt=f16)
            wk = load([D, NH * D], wk_h[:, :], nc.vector.tensor_copy, "wk",
                      dt=f16, sdt=f16)
            wv = load([D, NH * D], wv_h[:, :], nc.vector.tensor_copy, "wv",
                      dt=f16, sdt=f16)
            wo = load([D, NH * D], wo_h[:, :], nc.vector.tensor_copy, "wo",
                      dt=f16, sdt=f16)

            # DVE-consumed: staged via ACT
            tsj_ap = tsj_h[:, :]
            tsj_b = bass.AP(
                tensor=tsj_ap.tensor, offset=tsj_ap.offset,
                ap=[[0, P], list(tsj_ap.ap[-1])],
            )
            tsj = load([P, S], tsj_b, nc.scalar.copy, "tsj")
            tsi = load([P, NT], tsi_h[:, :], nc.scalar.copy, "tsi")

            # --- projections: qt, kt [128(2h*64d), S]; v2 [128s, NT*128(2h*64e)] ---
            qt = consts.tile([P, S], f16)
            kt = consts.tile([P, S], f16)
            v2 = consts.tile([P, NT * P], f16)
            with tc.tile_pool(name="psetup", bufs=2, space="PSUM") as psetup:
                for w8, dst in ((wq, qt), (wk, kt)):
                    for sc in range(S // 1024):
                        pq = psetup.tile([P, 1024], f32, tag="ps")
                        for j in (0, 512):
                            nc.tensor.matmul(
                                pq[:, j:j + 512], lhsT=w8,
                                rhs=xT[:, 1024 * sc + j:1024 * sc + j + 512],
                                start=True, stop=True)
                        nc.scalar.copy(dst[:, 1024 * sc:1024 * (sc + 1)], pq)
                for g in range(2):  # 8 s-tiles per round -> one [P,1024] psum
                    pv = psetup.tile([P, 8 * P], f32, tag="ps")
                    for q in range(8):
                        st = 8 * g + q
                        nc.tensor.matmul(pv[:, q * P:(q + 1) * P],
                                         lhsT=xT[:, P * st:P * (st + 1)],
                                         rhs=wv, start=True, stop=True)
                    nc.scalar.copy(v2[:, g * 8 * P:(g + 1) * 8 * P], pv)

            # --- main pipeline ---
            d2all = consts.tile([P, NT * NH], f32, tag="d2all")

            work = ctx.enter_context(tc.tile_pool(name="work", bufs=3))
            small = ctx.enter_context(tc.tile_pool(name="small", bufs=6))
            ppe = ctx.enter_context(tc.tile_pool(name="ppe", bufs=2,
                                                 space="PSUM"))
            ppt = ctx.enter_context(tc.tile_pool(name="ppt", bufs=2,
                                                 space="PSUM"))
            pprT = ctx.enter_context(tc.tile_pool(name="pprT", bufs=1,
                                                  space="PSUM"))
            ppo = ctx.enter_context(tc.tile_pool(name="ppo", bufs=1,
                                                 space="PSUM"))

            CH = 1024  # QK chunk width (PSUM tile)

            for ti in range(NT):
                W = P * (ti + 1)
                nch = (W + CH - 1) // CH
                po2 = ppo.tile([P, NH, D], f32)
                for h in range(NH):
                    hs = slice(D * h, D * (h + 1))
                    # QK scores -> PSUM chunks -> es fp16 in SBUF
                    es = work.tile([P, S], f16, tag="es")
                    qrow = qt[hs, P * ti:P * (ti + 1)]
                    cpy = nc.vector.tensor_copy if h == 0 else nc.scalar.copy
                    for c in range(nch):
                        lo, hi = CH * c, min(W, CH * (c + 1))
                        pe = ppe.tile([P, CH], f32, tag="pe")
                        j0 = lo
                        while j0 < hi:
                            j1 = min(hi, j0 + 512)
                            last = j1 == W
                            if last:
                                # pre-write the causal mask into PSUM, then
                                # accumulate the QK product on top
                                cw = j1 - j0
                                nc.tensor.matmul(
                                    pe[:, j0 - lo:j1 - lo], lhsT=identf,
                                    rhs=mask[:, 512 - cw:512],
                                    start=True, stop=False)
                            nc.tensor.matmul(pe[:, j0 - lo:j1 - lo],
                                             lhsT=qrow, rhs=kt[hs, j0:j1],
                                             start=not last, stop=True)
                            j0 = j1
                        cpy(es[:, lo:hi], pe[:, :hi - lo])

                    # ex1 = exp(es), den via accum (one full-width instr)
                    ex1 = work.tile([P, S], f16, tag="ex1")
                    den = small.tile([P, 1], f32, tag="den")
                    nc.scalar.activation(ex1[:, :W], es[:, :W], Act.Exp,
                                         accum_out=den)
                    rden = small.tile([P, 1], f32, tag="rden")
                    nc.vector.reciprocal(rden, den)

                    # decay' = ((t_i'-t_j')*(1-cum*rden) - H)^2  [custom DVE]
                    dz = work.tile([P, S], f16, tag="dz")
                    nc.vector._custom_dve(
                        fused_op, out=dz[:, :W], in0=tsj[:, :W],
                        in1=ex1[:, :W], s0=tsi[:, ti:ti + 1], s1=rden,
                        imm2=h_imm)

                    # w = es * decay'   (fp16 2x)
                    w = work.tile([P, S], f16, tag="w")
                    nc.vector.tensor_mul(w[:, :W], es[:, :W], dz[:, :W])

                    # u = exp(w), den2 accumulated straight into d2all
                    u = work.tile([P, S], f16, tag="u")
                    nc.scalar.activation(u[:, :W], w[:, :W], Act.Exp,
                                         accum_out=d2all[:, ti * NH + h:
                                                         ti * NH + h + 1])

                    # AV: transpose u blocks (8 per PSUM tile), accumulate
                    prT = pprT.tile([D, P], f32, tag="prT")
                    njb = ti + 1
                    for g0 in range(0, njb, 8):
                        gn = min(8, njb - g0)
                        uT8 = small.tile([P, 8 * P], f16, tag="uT8")
                        pt = ppt.tile([P, 8 * P], f16, tag="pt")
                        for q in range(gn):
                            nc.tensor.transpose(
                                pt[:, q * P:(q + 1) * P],
                                u[:, (g0 + q) * P:(g0 + q + 1) * P], identf)
                        nc.vector.tensor_copy(uT8[:, :gn * P], pt[:, :gn * P])
                        for q in range(gn):
                            jb = g0 + q
                            nc.tensor.matmul(
                                prT,
                                lhsT=v2[:, jb * P + D * h: jb * P + D * (h + 1)],
                                rhs=uT8[:, q * P:(q + 1) * P],
                                start=(jb == 0), stop=(jb == ti))
                    rT = small.tile([D, P], f16, tag="rT")
                    nc.vector.tensor_copy(rT, prT)
                    nc.tensor.matmul(po2[:, h, :], lhsT=rT, rhs=wo[:, hs],
                                     start=True, stop=True)

                # unnormalized head outputs -> SBUF -> DRAM
                ys = small.tile([P, NH * D], f32, tag="ys")
                nc.vector.tensor_copy(ys, po2[:, :, :])
                nc.sync.dma_start(out=y_h[P * ti:P * (ti + 1), :], in_=ys)
            nc.sync.dma_start(out=d2_h[:, :], in_=d2all)

    if not nc.is_finalized():
        nc.finalize()
    return nc


_NC_CACHE = {}

KERNEL_FLAGS = {}


def _get_nc():
    key = tuple(sorted(KERNEL_FLAGS.items()))
    if key not in _NC_CACHE:
        _NC_CACHE[key] = build_nc(**KERNEL_FLAGS)
    return _NC_CACHE[key]


def make_in_maps(inputs, timestamp, wQ, wK, wV, wO, theta):
    x = np.asarray(inputs, np.float32)
    t64 = np.asarray(timestamp).astype(np.float64)
    wQ = np.asarray(wQ, np.float32)
    wK = np.asarray(wK, np.float32)
    wV = np.asarray(wV, np.float32)
    wO = np.asarray(wO, np.float32)
    th2 = float(np.float64(np.asarray(theta, np.float64)[0, 0]) ** 2)
    sc = th2 * float(np.sqrt(C2P))
    tp = (t64 * sc).astype(np.float32)  # prescaled timestamps

    in_maps = []
    for c in range(NCORES):
        b = c // 4
        h0 = NH * (c % 4)
        in_maps.append({
            "xT": np.ascontiguousarray(x[b].T.astype(np.float16)),
            "tsj": np.ascontiguousarray(tp[b][None, :]),
            "tsi": np.ascontiguousarray(tp[b].reshape(NT, P).T),
            "wq": np.ascontiguousarray(
                (np.concatenate([wQ[h0], wQ[h0 + 1]], axis=1)
                 * 0.125).astype(np.float16)),
            "wk": np.ascontiguousarray(np.concatenate(
                [wK[h0], wK[h0 + 1]], axis=1).astype(np.float16)),
            "wv": np.ascontiguousarray(np.concatenate(
                [wV[h0], wV[h0 + 1]], axis=1).astype(np.float16)),
            "wo": np.ascontiguousarray(np.concatenate(
                [wO[h0 * D:(h0 + 1) * D], wO[(h0 + 1) * D:(h0 + 2) * D]],
                axis=1).astype(np.float16)),
        })
    return in_maps


def kernel(inputs, timestamp, wQ, wK, wV, wO, theta, _trace=False,
           _trace_kwargs=None):
    _import_concourse()
    from concourse.bass_utils import run_bass_kernel_spmd

    nc = _get_nc()
    in_maps = make_in_maps(inputs, timestamp, wQ, wK, wV, wO, theta)
    res = run_bass_kernel_spmd(
        nc, in_maps, list(range(NCORES)),
        trace=_trace, **(_trace_kwargs or {}),
    )
    out = np.zeros((B, S, D), np.float32)
    for c in range(NCORES):
        yp = res.results[c]["y"]  # [S, NH*D] unnormalized
        d2 = res.results[c]["d2"]  # [P, NT*NH]
        den2 = d2.reshape(P, NT, NH).transpose(1, 0, 2).reshape(S, NH)
        out[c // 4] += (yp[:, :D] / den2[:, 0:1]
                        + yp[:, D:] / den2[:, 1:2])
    if _trace:
        return out, res
    return out


if __name__ == "__main__":
    nc = build_nc()
    print("built ok")
